# revision 5
# baseline (speedup 1.0000x reference)
"""Causal self-attention (B=2,T=2048,C=1024,H=16) on 8 trn2 NeuronCores.

Sharding: 2 heads per core (head/tensor parallel on w_attn columns and
w_proj rows); each core computes a full-shape partial of the output
projection; host sums the 8 partials.

Self-contained: hardcodes all shapes; no sibling imports.
"""
import sys

for _p in ("/opt/trn_rl_repo", "/root/.axon_site/_ro/trn_rl_repo"):
    if _p not in sys.path:
        sys.path.append(_p)

import numpy as np
import ml_dtypes

B, T, C, H = 2, 2048, 1024, 16
Dh = C // H          # 64
NCORES = 8
HPC = H // NCORES    # 2 heads per core
BT = B * T           # 4096
NQT = T // 512       # 4 q-tiles of 512 per batch
NKT = T // 128       # 16 k-tiles of 128 per batch
SCALE = 1.0 / float(np.sqrt(Dh))

BF16 = ml_dtypes.bfloat16

_CACHE = {}


def _build_nc():
    import concourse.mybir as mybir
    import concourse.tile as tile
    from concourse import bacc

    dt = mybir.dt
    nc = bacc.Bacc("TRN2", target_bir_lowering=False, debug=False,
                   num_devices=NCORES)

    xt = nc.dram_tensor("xt", [C, BT], dt.bfloat16, kind="ExternalInput").ap()
    wqk = nc.dram_tensor("wqk", [C, 256], dt.bfloat16, kind="ExternalInput").ap()
    wv = nc.dram_tensor("wv", [C, 128], dt.bfloat16, kind="ExternalInput").ap()
    wp = nc.dram_tensor("wp", [128, C], dt.bfloat16, kind="ExternalInput").ap()
    cc = nc.dram_tensor("cc", [128, T], dt.bfloat16, kind="ExternalInput").ap()
    ss = nc.dram_tensor("ss", [128, T], dt.float32, kind="ExternalInput").ap()
    pswp = nc.dram_tensor("pswp", [128, 128], dt.bfloat16, kind="ExternalInput").ap()
    out = nc.dram_tensor("out", [BT, C], dt.float32, kind="ExternalOutput").ap()

    EXP = mybir.ActivationFunctionType.Exp

    with tile.TileContext(nc) as tc:
        with (
            tc.tile_pool(name="const", bufs=1) as constp,
            tc.tile_pool(name="xtp", bufs=2) as xtp,
            tc.tile_pool(name="qk", bufs=1) as qkp,
            tc.tile_pool(name="stg", bufs=4) as stg,
            tc.tile_pool(name="expp", bufs=6) as expp,
            tc.tile_pool(name="dnp", bufs=4) as dnp,
            tc.tile_pool(name="outp", bufs=4) as outp,
            tc.tile_pool(name="psA", bufs=4, space="PSUM") as psA,
            tc.tile_pool(name="psB", bufs=3, space="PSUM") as psB,
            tc.tile_pool(name="psV", bufs=1, space="PSUM") as psV,
        ):
            # ---- constants ----
            wqk_sb = [constp.tile([128, 256], dt.bfloat16, tag=f"wqk{k}", name=f"wqk_sb{k}")
                      for k in range(8)]
            wv_sb = [constp.tile([128, 128], dt.bfloat16, tag=f"wv{k}", name=f"wv_sb{k}")
                     for k in range(8)]
            for k in range(8):
                nc.sync.dma_start(out=wqk_sb[k][:], in_=wqk[k * 128:(k + 1) * 128, :])
                nc.sync.dma_start(out=wv_sb[k][:], in_=wv[k * 128:(k + 1) * 128, :])
            wp_sb = constp.tile([128, C], dt.bfloat16, tag="wp")
            nc.sync.dma_start(out=wp_sb[:], in_=wp)
            cc_sb = constp.tile([128, T], dt.bfloat16, tag="cc")
            nc.sync.dma_start(out=cc_sb[:], in_=cc)
            ss_sb = constp.tile([128, T], dt.float32, tag="ss")
            nc.sync.dma_start(out=ss_sb[:], in_=ss)
            pswp_sb = constp.tile([128, 128], dt.bfloat16, tag="pswp")
            nc.sync.dma_start(out=pswp_sb[:], in_=pswp)

            # persistent per-batch tensors
            qT = [qkp.tile([128, T], dt.bfloat16, tag=f"qT{b}", name=f"qT{b}") for b in range(B)]
            kT = [qkp.tile([128, T], dt.bfloat16, tag=f"kT{b}", name=f"kT{b}") for b in range(B)]
            vsb = [qkp.tile([128, 16 * 130], dt.bfloat16, tag=f"v{b}", name=f"vsb{b}")
                   for b in range(B)]
            yTn = [qkp.tile([128, T], dt.bfloat16, tag=f"y{b}", name=f"yTn{b}") for b in range(B)]

            # ---- projections + rope, per batch ----
            for b in range(B):
                xt_b = [xtp.tile([128, T], dt.bfloat16, tag=f"xt{k}", name=f"xt_b{k}")
                        for k in range(8)]
                for k in range(8):
                    nc.sync.dma_start(
                        out=xt_b[k][:], in_=xt[k * 128:(k + 1) * 128,
                                               b * T:(b + 1) * T])
                # v_aug ones columns: pre-fill whole tile, copies overwrite rest
                nc.vector.memset(vsb[b][:], 1.0)

                # q,k projection (transposed layout) + rope
                for m in range(2):           # 0 = q, 1 = k
                    dest = qT[b] if m == 0 else kT[b]
                    for n in range(NQT):     # 512-wide t chunks
                        tsl = slice(n * 512, (n + 1) * 512)
                        xp = psA.tile([128, 512], dt.float32, tag="psA")
                        for k in range(8):
                            nc.tensor.matmul(
                                out=xp[:], lhsT=wqk_sb[k][:, m * 128:(m + 1) * 128],
                                rhs=xt_b[k][:, tsl],
                                start=(k == 0), stop=(k == 7))
                        xsb = stg.tile([128, 512], dt.bfloat16, tag="xsb")
                        nc.vector.tensor_copy(out=xsb[:], in_=xp[:])
                        xs = psB.tile([128, 512], dt.float32, tag="psB")
                        nc.tensor.matmul(out=xs[:], lhsT=pswp_sb[:], rhs=xsb[:],
                                         start=True, stop=True)
                        r1 = stg.tile([128, 512], dt.bfloat16, tag="r1")
                        nc.vector.tensor_mul(out=r1[:], in0=xsb[:],
                                             in1=cc_sb[:, tsl])
                        r2 = stg.tile([128, 512], dt.bfloat16, tag="r2")
                        nc.vector.tensor_mul(out=r2[:], in0=xs[:],
                                             in1=ss_sb[:, tsl])
                        nc.vector.tensor_add(out=dest[:, tsl], in0=r1[:], in1=r2[:])

                # v projection (natural layout) + v_aug assembly
                for rt in range(NKT):        # 128-row t chunks
                    vp = psV.tile([128, 128], dt.float32, tag="psV")
                    for k in range(8):
                        nc.tensor.matmul(
                            out=vp[:], lhsT=xt_b[k][:, rt * 128:(rt + 1) * 128],
                            rhs=wv_sb[k][:], start=(k == 0), stop=(k == 7))
                    for h in range(HPC):
                        nc.vector.tensor_copy(
                            out=vsb[b][:, rt * 130 + h * 65:rt * 130 + h * 65 + 64],
                            in_=vp[:, h * 64:h * 64 + 64])

            # ---- attention, per (batch, q-tile) ----
            for b in range(B):
                for qi in range(NQT):
                    tmax = 4 * qi + 3
                    yt = [psB.tile([65, 512], dt.float32, tag="psB", name=f"yt{h}")
                          for h in range(HPC)]
                    for t in range(tmax + 1):
                        p = t - 4 * qi           # >=0 on diagonal k-tiles
                        j0 = 128 * p if p > 0 else 0
                        exps = []
                        for h in range(HPC):
                            sc = psA.tile([128, 512], dt.float32, tag="psA")
                            nc.tensor.matmul(
                                out=sc[:],
                                lhsT=kT[b][h * 64:(h + 1) * 64,
                                           t * 128:(t + 1) * 128],
                                rhs=qT[b][h * 64:(h + 1) * 64,
                                          qi * 512:(qi + 1) * 512],
                                start=True, stop=True)
                            ex = expp.tile([128, 512], dt.bfloat16, tag="ex")
                            nc.scalar.activation(out=ex[:, j0:512],
                                                 in_=sc[:, j0:512],
                                                 func=EXP, scale=SCALE)
                            if p >= 0:
                                # zero strictly-above-diagonal of this k-tile
                                nc.gpsimd.affine_select(
                                    out=ex[:, j0:j0 + 128],
                                    in_=ex[:, j0:j0 + 128],
                                    compare_op=mybir.AluOpType.is_ge,
                                    fill=0.0, base=0,
                                    pattern=[[1, 128]], channel_multiplier=-1)
                            exps.append(ex)
                        for h in range(HPC):
                            nc.tensor.matmul(
                                out=yt[h][0:65, j0:512],
                                lhsT=vsb[b][:, t * 130 + h * 65:t * 130 + h * 65 + 65],
                                rhs=exps[h][:, j0:512],
                                start=(t == 0), stop=(t == tmax))
                    # denominators + normalize
                    for h in range(HPC):
                        dn = dnp.tile([1, 512], dt.float32, tag="dn")
                        nc.vector.reciprocal(out=dn[:], in_=yt[h][64:65, :])
                        bc = dnp.tile([64, 512], dt.float32, tag="bc")
                        nc.gpsimd.partition_broadcast(bc[:], dn[:])
                        nc.vector.tensor_mul(
                            out=yTn[b][h * 64:(h + 1) * 64,
                                       qi * 512:(qi + 1) * 512],
                            in0=yt[h][0:64, :], in1=bc[:])

            # ---- output projection ----
            for b in range(B):
                for rt in range(NKT):
                    for ct in range(2):
                        op = psA.tile([128, 512], dt.float32, tag="psA")
                        nc.tensor.matmul(
                            out=op[:],
                            lhsT=yTn[b][:, rt * 128:(rt + 1) * 128],
                            rhs=wp_sb[:, ct * 512:(ct + 1) * 512],
                            start=True, stop=True)
                        og = outp.tile([128, 512], dt.float32, tag="og")
                        if (rt + ct) % 2 == 0:
                            nc.vector.tensor_copy(out=og[:], in_=op[:])
                        else:
                            nc.scalar.copy(out=og[:], in_=op[:])
                        nc.sync.dma_start(
                            out=out[b * T + rt * 128:b * T + (rt + 1) * 128,
                                    ct * 512:(ct + 1) * 512],
                            in_=og[:])
    nc.compile()
    return nc


def get_nc():
    if "nc" not in _CACHE:
        _CACHE["nc"] = _build_nc()
    return _CACHE["nc"]


def make_in_maps(x, w_attn, w_proj, freqs_cos, freqs_sin):
    x = np.asarray(x, dtype=np.float32)
    w_attn = np.asarray(w_attn, dtype=np.float32)
    w_proj = np.asarray(w_proj, dtype=np.float32)
    freqs_cos = np.asarray(freqs_cos, dtype=np.float32)
    freqs_sin = np.asarray(freqs_sin, dtype=np.float32)

    xt = np.ascontiguousarray(x.reshape(BT, C).T).astype(BF16)

    # rope tables: partition layout per 64-d head block = [32 even | 32 odd]
    cos_t = freqs_cos.T.astype(np.float32)          # [32, T]
    sin_t = freqs_sin.T.astype(np.float32)
    cc = np.concatenate([cos_t, cos_t, cos_t, cos_t], axis=0).astype(BF16)
    ss = np.concatenate([-sin_t, sin_t, -sin_t, sin_t], axis=0).astype(np.float32)

    # 32-block swap permutation (lhsT; symmetric)
    pswp = np.zeros((128, 128), dtype=np.float32)
    for i in range(128):
        pswp[i, (i // 32 ^ 1) * 32 + i % 32] = 1.0
    pswp = pswp.astype(BF16)

    perm = np.concatenate([np.arange(0, Dh, 2), np.arange(1, Dh, 2)])  # [64]

    in_maps = []
    for c in range(NCORES):
        cols = []
        for off in (0, C):                          # q then k sections
            for h in (HPC * c, HPC * c + 1):
                cols.append(off + h * Dh + perm)
        wqk_c = w_attn[:, np.concatenate([np.concatenate(cols[0:2]),
                                          np.concatenate(cols[2:4])])]
        wv_c = w_attn[:, 2 * C + HPC * c * Dh: 2 * C + HPC * (c + 1) * Dh]
        wp_c = w_proj[HPC * c * Dh: HPC * (c + 1) * Dh, :]
        in_maps.append({
            "xt": xt,
            "wqk": np.ascontiguousarray(wqk_c).astype(BF16),
            "wv": np.ascontiguousarray(wv_c).astype(BF16),
            "wp": np.ascontiguousarray(wp_c).astype(BF16),
            "cc": cc,
            "ss": ss,
            "pswp": pswp,
        })
    return in_maps


def kernel(x, w_attn, w_proj, freqs_cos, freqs_sin):
    from concourse import bass_utils

    nc = get_nc()
    in_maps = make_in_maps(x, w_attn, w_proj, freqs_cos, freqs_sin)
    res = bass_utils.run_bass_kernel_spmd(
        nc, in_maps, core_ids=list(range(NCORES)), trace=False)
    acc = res.results[0]["out"].astype(np.float64)
    for c in range(1, NCORES):
        acc += res.results[c]["out"]
    return acc.astype(np.float32).reshape(B, T, C)


# revision 18
# speedup vs baseline: 29155.2254x; 29155.2254x over previous
"""Causal self-attention (B=2,T=2048,C=1024,H=16) on 8 trn2 NeuronCores.

Sharding: 2 heads per core (head/tensor parallel on w_attn columns and
w_proj rows); each core computes a full-shape partial of the output
projection (bf16); host sums the 8 partials in fp32.

Self-contained: hardcodes all shapes; no sibling imports.
"""
import sys

for _p in ("/opt/trn_rl_repo", "/root/.axon_site/_ro/trn_rl_repo"):
    if _p not in sys.path:
        sys.path.append(_p)

import numpy as np
import ml_dtypes

B, T, C, H = 2, 2048, 1024, 16
Dh = C // H          # 64
NCORES = 8
HPC = H // NCORES    # 2 heads per core
BT = B * T           # 4096
QW = 1024            # q-tile width
NQT = T // QW        # 2 q-tiles per batch
NKT = T // 128       # 16 k-tiles of 128 per batch
KPQ = QW // 128      # 8 k-tiles per q-tile width
SCALE = 1.0 / float(np.sqrt(Dh))
NEG = -30000.0       # causal mask additive constant (pre-scale)

BF16 = ml_dtypes.bfloat16

_CACHE = {}


def _build_nc():
    import concourse.mybir as mybir
    import concourse.tile as tile
    from concourse import bacc

    dt = mybir.dt
    nc = bacc.Bacc("TRN2", target_bir_lowering=False, debug=False,
                   num_devices=NCORES)

    xt = nc.dram_tensor("xt", [C, BT], dt.bfloat16, kind="ExternalInput").ap()
    wqk = nc.dram_tensor("wqk", [C, 256], dt.bfloat16, kind="ExternalInput").ap()
    wv = nc.dram_tensor("wv", [C, 128], dt.bfloat16, kind="ExternalInput").ap()
    wp = nc.dram_tensor("wp", [128, C], dt.bfloat16, kind="ExternalInput").ap()
    cc = nc.dram_tensor("cc", [128, T], dt.bfloat16, kind="ExternalInput").ap()
    ss = nc.dram_tensor("ss", [128, T], dt.float32, kind="ExternalInput").ap()
    pswp = nc.dram_tensor("pswp", [128, 128], dt.bfloat16, kind="ExternalInput").ap()
    atri = nc.dram_tensor("atri", [128, 128], dt.bfloat16, kind="ExternalInput").ap()
    bdg = nc.dram_tensor("bdg", [128, 128], dt.bfloat16, kind="ExternalInput").ap()
    out = nc.dram_tensor("out", [BT, C], dt.bfloat16, kind="ExternalOutput").ap()

    EXP = mybir.ActivationFunctionType.Exp

    with tile.TileContext(nc) as tc:
        with (
            tc.tile_pool(name="const", bufs=1) as constp,
            tc.tile_pool(name="xtp", bufs=2) as xtp,
            tc.tile_pool(name="qk", bufs=1) as qkp,
            tc.tile_pool(name="stg", bufs=3) as stg,
            tc.tile_pool(name="expp", bufs=4) as expp,
            tc.tile_pool(name="dnp", bufs=2) as dnp,
            tc.tile_pool(name="outp", bufs=6) as outp,
        ):
            # ---- constants (batched single DMAs) ----
            wqk_all = constp.tile([128, 8 * 256], dt.bfloat16, tag="wqka")
            nc.sync.dma_start(
                out=wqk_all[:].rearrange("p (ko c) -> p ko c", ko=8),
                in_=wqk.rearrange("(ko p) c -> p ko c", p=128))
            wv_all = constp.tile([128, 8 * 128], dt.bfloat16, tag="wva")
            nc.sync.dma_start(
                out=wv_all[:].rearrange("p (ko c) -> p ko c", ko=8),
                in_=wv.rearrange("(ko p) c -> p ko c", p=128))
            wqk_sb = [wqk_all[:, k * 256:(k + 1) * 256] for k in range(8)]
            wv_sb = [wv_all[:, k * 128:(k + 1) * 128] for k in range(8)]
            pswp_sb = constp.tile([128, 128], dt.bfloat16, tag="pswp")
            nc.sync.dma_start(out=pswp_sb[:], in_=pswp)
            wp_sb = constp.tile([128, C], dt.bfloat16, tag="wp")
            cc_sb = constp.tile([128, T], dt.bfloat16, tag="cc")
            ss_sb = constp.tile([128, T], dt.float32, tag="ss")
            atri_sb = constp.tile([128, 128], dt.bfloat16, tag="atri")
            bdg_sb = constp.tile([128, 128], dt.bfloat16, tag="bdg")

            def late_const_dmas():
                nc.sync.dma_start(out=cc_sb[:], in_=cc)
                nc.sync.dma_start(out=ss_sb[:], in_=ss)
                nc.sync.dma_start(out=atri_sb[:], in_=atri)
                nc.sync.dma_start(out=bdg_sb[:], in_=bdg)
                nc.sync.dma_start(out=wp_sb[:], in_=wp)

            # persistent per-batch tensors
            qT = [qkp.tile([128, T], dt.bfloat16, tag=f"qT{b}", name=f"qT{b}")
                  for b in range(B)]
            kT = [qkp.tile([128, T], dt.bfloat16, tag=f"kT{b}", name=f"kT{b}")
                  for b in range(B)]
            vsb = [qkp.tile([128, 16 * 130], dt.bfloat16, tag=f"v{b}",
                            name=f"vsb{b}") for b in range(B)]
            yTn = [qkp.tile([128, T], dt.bfloat16, tag=f"y{b}", name=f"yTn{b}")
                   for b in range(B)]

            # ---- projections + rope, per batch (own psum pools) ----
            proj_pools = tc.tile_pool(name="psA", bufs=2, space="PSUM")
            psA = proj_pools.__enter__()
            proj_pools2 = tc.tile_pool(name="psB", bufs=2, space="PSUM")
            psB = proj_pools2.__enter__()
            for b in range(B):
                # xt chunks [128, T] per k, DMA'd k-major so the k-outer
                # accumulation below can start after the first chunk
                xt_b = [xtp.tile([128, T], dt.bfloat16, tag=f"xt{k}",
                                 name=f"xt_b{k}") for k in range(8)]
                for k in range(8):
                    nc.sync.dma_start(
                        out=xt_b[k][:], in_=xt[k * 128:(k + 1) * 128,
                                               b * T:(b + 1) * T])
                if b == 0:
                    late_const_dmas()
                # v_aug ones columns: pre-fill whole tile, copies overwrite rest
                nc.vector.memset(vsb[b][:], 1.0)

                # q,k projection (transposed layout), k-outer; rope after
                for m in range(2):           # 0 = q, 1 = k
                    dest = qT[b] if m == 0 else kT[b]
                    xps = [psA.tile([128, QW], dt.float32, tag="psA",
                                    name=f"xp{n}") for n in range(NQT)]
                    for k in range(8):
                        for n in range(NQT):
                            for half in range(2):
                                hs = slice(half * 512, (half + 1) * 512)
                                nc.tensor.matmul(
                                    out=xps[n][:, hs],
                                    lhsT=wqk_sb[k][:, m * 128:(m + 1) * 128],
                                    rhs=xt_b[k][:, n * QW + half * 512:
                                                n * QW + (half + 1) * 512],
                                    start=(k == 0), stop=(k == 7))
                    for n in range(NQT):     # rope per 1024-wide chunk
                        tsl = slice(n * QW, (n + 1) * QW)
                        xp = xps[n]
                        xsb = stg.tile([128, QW], dt.bfloat16, tag="xsb",
                                       name="xsb")
                        nc.scalar.copy(out=xsb[:], in_=xp[:])
                        xs = psB.tile([128, QW], dt.float32, tag="psB", name="xs")
                        for half in range(2):
                            hs = slice(half * 512, (half + 1) * 512)
                            nc.tensor.matmul(out=xs[:, hs], lhsT=pswp_sb[:],
                                             rhs=xsb[:, hs],
                                             start=True, stop=True)
                        r1 = stg.tile([128, QW], dt.bfloat16, tag="r1",
                                      name="r1")
                        nc.vector.tensor_mul(out=r1[:], in0=xsb[:],
                                             in1=cc_sb[:, tsl])
                        r2 = stg.tile([128, QW], dt.bfloat16, tag="r2",
                                      name="r2")
                        nc.vector.tensor_mul(out=r2[:], in0=xs[:],
                                             in1=ss_sb[:, tsl])
                        nc.vector.tensor_add(out=dest[:, tsl], in0=r1[:],
                                             in1=r2[:])

                # v projection (natural layout) + v_aug assembly
                for rt in range(NKT):        # 128-row t chunks
                    vp = psA.tile([128, 128], dt.float32, tag="psA", name="vp")
                    for k in range(8):
                        nc.tensor.matmul(
                            out=vp[:],
                            lhsT=xt_b[k][:, rt * 128:(rt + 1) * 128],
                            rhs=wv_sb[k][:], start=(k == 0), stop=(k == 7))
                    for h in range(HPC):
                        nc.scalar.copy(
                            out=vsb[b][:, rt * 130 + h * 65:rt * 130 + h * 65 + 64],
                            in_=vp[:, h * 64:h * 64 + 64])

            proj_pools2.__exit__(None, None, None)
            proj_pools.__exit__(None, None, None)

            # ---- attention per (batch, q-tile), then fused out-proj ----
            def emit_scores(psSc, b, qi, t):
                """Scores (fp32 psum) + additive causal mask, both heads."""
                p = t - KPQ * qi
                j0 = 128 * p if p > 0 else 0
                mask_bank = j0 // 512 if p >= 0 else -1
                scs = []
                for h in range(HPC):
                    sc = psSc.tile([128, QW], dt.float32, tag="sc", name="sc")
                    for half in range(2):
                        lo, hi = half * 512, (half + 1) * 512
                        lo = max(lo, j0)
                        if lo >= hi:
                            continue
                        nc.tensor.matmul(
                            out=sc[:, lo:hi],
                            lhsT=kT[b][h * 64:(h + 1) * 64,
                                       t * 128:(t + 1) * 128],
                            rhs=qT[b][h * 64:(h + 1) * 64,
                                      qi * QW + lo:qi * QW + hi],
                            start=True, stop=(half != mask_bank))
                    if p >= 0:
                        # sc[:, j0:j0+128] += NEG * [part > j']
                        nc.tensor.matmul(
                            out=sc[:, j0:j0 + 128],
                            lhsT=atri_sb[:], rhs=bdg_sb[:],
                            start=False, stop=True)
                    scs.append(sc)
                return scs

            def emit_expav(psYt, b, qi, t, scs, yt):
                p = t - KPQ * qi
                j0 = 128 * p if p > 0 else 0
                tmax = min(KPQ * qi + KPQ - 1, NKT - 1)
                for h in range(HPC):
                    ex = expp.tile([128, QW], dt.bfloat16, tag="ex", name="ex")
                    nc.scalar.activation(out=ex[:, j0:QW], in_=scs[h][:, j0:QW],
                                         func=EXP, scale=SCALE)
                    for half in range(2):
                        lo, hi = half * 512, (half + 1) * 512
                        lo = max(lo, j0)
                        if lo >= hi:
                            continue
                        # last writer of this bank closes its accum group
                        last_t = tmax if half == 1 else min(KPQ * qi + 3, tmax)
                        nc.tensor.matmul(
                            out=yt[h][0:65, lo:hi],
                            lhsT=vsb[b][:, t * 130 + h * 65:t * 130 + h * 65 + 65],
                            rhs=ex[:, lo:hi],
                            start=(t == 0), stop=(t == last_t))

            with (
                tc.tile_pool(name="psSc", bufs=2, space="PSUM") as psSc,
                tc.tile_pool(name="psYt", bufs=2, space="PSUM") as psYt,
            ):
                fillers = []

                def emit_outproj_unit(b, rt, ct):
                    def unit():
                        op = psSc.tile([128, 512], dt.float32, tag="sc",
                                       name="op")
                        nc.tensor.matmul(
                            out=op[:],
                            lhsT=yTn[b][:, rt * 128:(rt + 1) * 128],
                            rhs=wp_sb[:, ct * 512:(ct + 1) * 512],
                            start=True, stop=True)
                        og = outp.tile([128, 512], dt.bfloat16, tag="og",
                                       name="og")
                        nc.vector.tensor_copy(out=og[:], in_=op[:])
                        nc.sync.dma_start(
                            out=out[b * T + rt * 128:b * T + (rt + 1) * 128,
                                    ct * 512:(ct + 1) * 512],
                            in_=og[:])
                    return unit

                blocks = [(b, qi) for b in range(B) for qi in range(NQT)]
                yts = {}
                scs_prev = emit_scores(psSc, *blocks[0], 0)
                for bi, (b, qi) in enumerate(blocks):
                    tmax = min(KPQ * qi + KPQ - 1, NKT - 1)
                    yt = [psYt.tile([65, QW], dt.float32, tag="yt",
                                    name=f"yt{h}") for h in range(HPC)]
                    for t in range(tmax + 1):
                        if t < tmax:
                            scs_next = emit_scores(psSc, b, qi, t + 1)
                        elif bi + 1 < len(blocks):
                            scs_next = emit_scores(psSc, *blocks[bi + 1], 0)
                        else:
                            scs_next = None
                        emit_expav(psYt, b, qi, t, scs_prev, yt)
                        nfill = 2 if tmax < 8 else 1
                        if t > 0:
                            for _ in range(nfill):
                                if fillers:
                                    fillers.pop(0)()
                        scs_prev = scs_next
                    # denominators + normalize
                    for h in range(HPC):
                        dn = dnp.tile([1, QW], dt.float32, tag="dn",
                                      name="dn")
                        nc.vector.reciprocal(out=dn[:], in_=yt[h][64:65, :])
                        bc = dnp.tile([64, QW], dt.float32, tag="bc",
                                      name="bc")
                        nc.gpsimd.partition_broadcast(bc[:], dn[:])
                        nc.vector.tensor_mul(
                            out=yTn[b][h * 64:(h + 1) * 64,
                                       qi * QW:(qi + 1) * QW],
                            in0=yt[h][0:64, :], in1=bc[:])
                    # queue this q-tile's out-projection as PE filler
                    for rt in range(qi * QW // 128, (qi + 1) * QW // 128):
                        for ct in range(2):
                            fillers.append(emit_outproj_unit(b, rt, ct))
                for f in fillers:
                    f()
    nc.compile()
    return nc


def get_nc():
    if "nc" not in _CACHE:
        _CACHE["nc"] = _build_nc()
    return _CACHE["nc"]


def make_in_maps(x, w_attn, w_proj, freqs_cos, freqs_sin):
    x = np.asarray(x, dtype=np.float32)
    w_attn = np.asarray(w_attn, dtype=np.float32)
    w_proj = np.asarray(w_proj, dtype=np.float32)
    freqs_cos = np.asarray(freqs_cos, dtype=np.float32)
    freqs_sin = np.asarray(freqs_sin, dtype=np.float32)

    xt = np.ascontiguousarray(x.reshape(BT, C).T).astype(BF16)

    # rope tables: partition layout per 64-d head block = [32 even | 32 odd]
    cos_t = freqs_cos.T.astype(np.float32)          # [32, T]
    sin_t = freqs_sin.T.astype(np.float32)
    cc = np.concatenate([cos_t, cos_t, cos_t, cos_t], axis=0).astype(BF16)
    ss = np.concatenate([-sin_t, sin_t, -sin_t, sin_t], axis=0).astype(np.float32)

    # 32-block swap permutation (lhsT; symmetric)
    pswp = np.zeros((128, 128), dtype=np.float32)
    for i in range(128):
        pswp[i, (i // 32 ^ 1) * 32 + i % 32] = 1.0
    pswp = pswp.astype(BF16)

    # causal-mask rank decomposition: (atri^T @ bdg)[part, j] = NEG*[part > j]
    atri = np.triu(np.ones((128, 128), dtype=np.float32), k=1).astype(BF16)
    bdg = (NEG * np.eye(128, dtype=np.float32)).astype(BF16)

    perm = np.concatenate([np.arange(0, Dh, 2), np.arange(1, Dh, 2)])  # [64]

    in_maps = []
    for c in range(NCORES):
        cols = []
        for off in (0, C):                          # q then k sections
            for h in (HPC * c, HPC * c + 1):
                cols.append(off + h * Dh + perm)
        wqk_c = w_attn[:, np.concatenate([np.concatenate(cols[0:2]),
                                          np.concatenate(cols[2:4])])]
        wv_c = w_attn[:, 2 * C + HPC * c * Dh: 2 * C + HPC * (c + 1) * Dh]
        wp_c = w_proj[HPC * c * Dh: HPC * (c + 1) * Dh, :]
        in_maps.append({
            "xt": xt,
            "wqk": np.ascontiguousarray(wqk_c).astype(BF16),
            "wv": np.ascontiguousarray(wv_c).astype(BF16),
            "wp": np.ascontiguousarray(wp_c).astype(BF16),
            "cc": cc,
            "ss": ss,
            "pswp": pswp,
            "atri": atri,
            "bdg": bdg,
        })
    return in_maps


def kernel(x, w_attn, w_proj, freqs_cos, freqs_sin):
    from concourse import bass_utils

    nc = get_nc()
    in_maps = make_in_maps(x, w_attn, w_proj, freqs_cos, freqs_sin)
    res = bass_utils.run_bass_kernel_spmd(
        nc, in_maps, core_ids=list(range(NCORES)), trace=False)
    acc = res.results[0]["out"].astype(np.float32)
    for c in range(1, NCORES):
        acc += res.results[c]["out"].astype(np.float32)
    return acc.reshape(B, T, C)
